# revision 28
# baseline (speedup 1.0000x reference)
"""DIN-attention Trainium2 kernel.

out[b] = softmax_t(MLP(concat[q, k, q-k, q*k]) / sqrt(H), mask=t<len_b) @ keys[b]

Strategy (8-core data parallel over B, one shared SPMD program):
- Host sorts b by keys_length, deals round-robin to cores -> per-core slot s
  holds similar lengths on every core; per 16-slot sub-block, work is
  truncated to the sub-block max length (halves all work in expectation).
- MLP decomposition: din@W1 = q@Wq + k@Wk + (q*k)@Wqk with
  Wq=W1a+W1c, Wk=W1b-W1c, Wqk=W1d; the q-term enters the PSUM group via a
  stride-0 broadcast rhs matmul, so relu bias is just b1.
- Scores (M=1 matmuls) are packed 4-per-PSUM-bank at partitions {0,32,64,96},
  then redistributed into a [128, Tg] scores tile with a PER-SLOT COLUMN
  OFFSET of j*tsb (j = slot's index within its k-pack).  After the batched
  masked softmax + PE transpose, column r of the transposed attn tile holds
  slot r's weights at partitions [j*tsb, (j+1)*tsb) -- i.e. k pack members
  form a block-diagonal stationary operand.
- Output contraction: one matmul per k-pack: lhsT = [k*tsb, k] block-diag
  attn columns, rhs = host-packed native keys stacked k-per-partition-dim,
  out = [k, 128] rows = slots.  For tsb>128, 4 slots ride side-by-side in a
  512-wide moving operand instead (2 chunks accumulated).
"""

import os
import sys
from contextlib import ExitStack

for _p in ("/opt/trn_rl_repo",):
    if _p not in sys.path:
        sys.path.insert(0, _p)

os.environ.setdefault("CONCOURSE_ENABLE_LDW_OPT", "false")

import numpy as np
import ml_dtypes

import concourse.bass as bass
import concourse.tile as tile
from concourse import bacc, mybir
from concourse.masks import make_identity

F32 = mybir.dt.float32
BF16 = mybir.dt.bfloat16
A = mybir.AluOpType
AF = mybir.ActivationFunctionType

B, T, H = 2048, 200, 128
H1, H2 = 80, 40
NC = 8
SLOTS = B // NC          # 256 slots per core
SB = 16                  # slots per sub-block
NSB = SLOTS // SB        # 16 sub-blocks per core
GROUP_SBS = 4            # sub-blocks per softmax group
NGROUPS = NSB // GROUP_SBS
SCALE = float(1.0 / np.sqrt(np.float32(H)))
NEG = -1e9


def _roundup(x, m):
    return ((int(x) + m - 1) // m) * m


def _kpack(t):
    """Slots packed per final matmul (vertical stacking).  Capped at 4: the
    scores-redistribute DMA must be split per pack-member (SBUF access
    patterns cannot mix partition and byte steps), so large k only adds
    tiny DMAs."""
    if t > 128:
        return 1
    k = 128 // t
    for cand in (4, 2, 1):
        if k >= cand:
            return cand
    return 1


def make_plan(keys_length):
    """Global plan shared by all cores: slot assignment + per-sub-block T."""
    order = np.argsort(keys_length, kind="stable")
    bmap = order.reshape(SLOTS, NC)          # [slot, core] -> b
    lens_slot = np.asarray(keys_length)[bmap]  # [slot, core]
    t_sbs, k_sbs, np_sbs, ext_sbs = [], [], [], []
    for sb in range(NSB):
        m = int(lens_slot[sb * SB:(sb + 1) * SB].max())
        t = min(T, _roundup(m, 8))
        t_sbs.append(t)
        k = _kpack(t)
        k_sbs.append(k)
        np_sbs.append(SB // k)
        ext_sbs.append(t if t > 128 else k * t)
    nchs = [max(1, -(-t // 128)) for t in t_sbs]
    kt_offs, off = [], 0
    for t in t_sbs:
        kt_offs.append(off)
        off += SB * t
    kt_w = off
    # kn (native keys) column layout per sub-block:
    #  t <= 128: [k*t rows, npacks*128 cols], pack p at cols p*128
    #  t > 128:  chunk0 [128 rows, 2048 cols] then chunk1 [t-128 rows, 2048]
    kn_offs, off = [], 0
    for sb in range(NSB):
        kn_offs.append(off)
        if t_sbs[sb] <= 128:
            off += np_sbs[sb] * 128
        else:
            off += 4096
    kn_w = off
    tgs = [max(ext_sbs[g * GROUP_SBS:(g + 1) * GROUP_SBS])
           for g in range(NGROUPS)]
    return dict(bmap=bmap, t_sbs=t_sbs, nchs=nchs, k_sbs=k_sbs, np_sbs=np_sbs,
                ext_sbs=ext_sbs, kt_offs=kt_offs, kt_w=kt_w,
                kn_offs=kn_offs, kn_w=kn_w, tgs=tgs)


def _row_of(ssb, k, npacks):
    """scores-row (within sub-block) of slot ssb.
    k>1: members of pack p=ssb//k land at rows j*npacks + p (pack members
    get an arithmetic column stride of npacks in the transposed-attn tile).
    k==1: the baseline quad permutation (from the one-DMA redistribute);
    wide pack p then uses atts columns [p::4]."""
    if k == 1:
        return 4 * (ssb % 4) + ssb // 4
    p, j = ssb // k, ssb % k
    return j * npacks + p


def _redis_dmas(k, t, sp, P, npk):
    """Per-DMA (src_off, dst_off, src_dims, dst_dims) for scratch->scores.
    scratch: slot ssb at (partition 32*(ssb%4), col (ssb//4)*t), pitch sp.
    scores dst: row _row_of(ssb), col (ssb%k)*t, pitch P.  Only dim 0 of an
    SBUF access pattern may step partitions, so the per-member column shift
    forces one DMA per (member j [, quad parity]) when k>1."""
    if k == 1:
        # baseline permutation: src (quad, colblock, t) -> sequential rows
        return [(0, 0, [[32 * sp, 4], [t, 4], [1, t]], [[P, 16], [1, t]])]
    if k == 2:
        # ssb = 2*(2*pa+pb) + j at (partition 32*(2*pb+j), col pa*t)
        return [(32 * (2 * pb + j) * sp, (j * 8 + pb) * P + j * t,
                 [[sp, 1], [1, 4 * t]], [[2 * P, 4], [1, t]])
                for j in range(2) for pb in range(2)]
    # k == 4: ssb = 4p + j at (partition 32*j, col p*t)
    return [(32 * j * sp, j * (4 * P + t),
             [[sp, 1], [1, 4 * t]], [[P, 4], [1, t]]) for j in range(4)]


SECTION_MARKS = []


def _mark(nc, label):
    SECTION_MARKS.append((len(nc.inst_map), label))


def build_body(ctx, tc, outs, ins, plan):
    nc = tc.nc
    SECTION_MARKS.clear()
    keysT_d, knat_d, queryT_d, lens_d = ins[:4]
    out_d, = outs
    t_sbs, nchs = plan["t_sbs"], plan["nchs"]
    k_sbs, np_sbs, ext_sbs = plan["k_sbs"], plan["np_sbs"], plan["ext_sbs"]
    kt_offs, kn_offs, tgs = plan["kt_offs"], plan["kn_offs"], plan["tgs"]

    singles = ctx.enter_context(tc.tile_pool(name="singles", bufs=1))
    kt_pool = ctx.enter_context(tc.tile_pool(name="kt", bufs=3))
    kn_pool = ctx.enter_context(tc.tile_pool(name="kn", bufs=GROUP_SBS + 1))
    qk_pool = ctx.enter_context(tc.tile_pool(name="qk", bufs=2))
    h1_pool = ctx.enter_context(tc.tile_pool(name="h1", bufs=3))
    h2_pool = ctx.enter_context(tc.tile_pool(name="h2", bufs=3))
    scr_pool = ctx.enter_context(tc.tile_pool(name="scr", bufs=3))
    grp_pool = ctx.enter_context(tc.tile_pool(name="grp", bufs=2))
    at_pool = ctx.enter_context(tc.tile_pool(name="at", bufs=4))
    ps1_pool = ctx.enter_context(tc.tile_pool(name="ps1", bufs=2, space="PSUM"))
    ps2_pool = ctx.enter_context(tc.tile_pool(name="ps2", bufs=2, space="PSUM"))
    psper_pool = ctx.enter_context(tc.tile_pool(name="psper", bufs=1, space="PSUM"))
    pst_pool = ctx.enter_context(tc.tile_pool(name="pst", bufs=1, space="PSUM"))

    # ---- constants (kt of the first-processed sub-block + qt first) ----
    sb0 = 0
    kts = {sb0: kt_pool.tile([H, SB * t_sbs[sb0]], BF16, tag="kt",
                             name=f"kt_{sb0}")}
    nc.sync.dma_start(kts[sb0][:],
                      keysT_d[:, kt_offs[sb0]:kt_offs[sb0] + SB * t_sbs[sb0]])
    qt = singles.tile([H, SLOTS], BF16, name="qt")
    nc.sync.dma_start(qt[:], queryT_d)
    wk = singles.tile([H, H1], BF16, name="wk")
    wqk = singles.tile([H, H1], BF16, name="wqk")
    wq = singles.tile([H, H1], BF16, name="wq")
    w2 = singles.tile([H1, H2], BF16, name="w2")
    wf = singles.tile([H2, 1], BF16, name="wf")
    b1c = singles.tile([H1, 1], F32, name="b1c")
    b2c = singles.tile([H2, 1], F32, name="b2c")
    wk_d, wqk_d, wq_d, w2_d, wf_d, b1_d, b2_d = ins[4:11]
    nc.sync.dma_start(wk[:], wk_d)
    nc.sync.dma_start(wqk[:], wqk_d)
    nc.sync.dma_start(wq[:], wq_d)
    nc.sync.dma_start(w2[:], w2_d)
    nc.sync.dma_start(wf[:], wf_d)
    nc.sync.dma_start(b1c[:], b1_d[:, None])
    nc.sync.dma_start(b2c[:], b2_d[:, None])
    lens = singles.tile([GROUP_SBS * SB, NGROUPS], F32, name="lens")
    nc.sync.dma_start(lens[:], lens_d)
    iota = singles.tile([128, T], F32, name="iota")
    nc.gpsimd.iota(iota[:], pattern=[[1, T]], base=0, channel_multiplier=0,
                   allow_small_or_imprecise_dtypes=True)
    # masks are static per group: build them early on the idle gpsimd
    masks = []
    for g in range(NGROUPS):
        mk = singles.tile([GROUP_SBS * SB, tgs[g]], F32, name=f"mask_{g}")
        nc.gpsimd.tensor_scalar(mk[:], iota[0:GROUP_SBS * SB, 0:tgs[g]],
                                lens[:, g:g + 1], None, op0=A.is_lt)
        masks.append(mk)
    identb = singles.tile([128, 128], BF16, name="identb")
    make_identity(nc, identb[:])
    zeros1 = singles.tile([1, 128], BF16, name="zeros1")
    nc.vector.memset(zeros1[:], 0.0)
    dummy512 = singles.tile([1, 512], BF16, name="dummy512")
    nc.vector.memset(dummy512[:], 0.0)
    # persistent, one-time-zeroed PSUM banks (manual double buffering)
    pss_t = [psper_pool.tile([128, 512], F32, tag="pssp0", name="pssp0")]
    pso_t = [psper_pool.tile([128, 512], F32, tag=f"psop{i}", name=f"psop{i}")
             for i in range(2)]
    for t_ in pss_t + pso_t:
        nc.tensor.matmul(t_[:], zeros1[:], dummy512[:], start=True, stop=True)

    qt_pitch = qt[:].ap[0][0]

    for g in range(NGROUPS):
        tg = tgs[g]
        gslots = GROUP_SBS * SB  # 128
        scores = grp_pool.tile([gslots, tg], F32, tag="scores", name=f"scores_g{g}")
        nc.vector.memset(scores[:], NEG)
        knats = {}
        for isb in range(GROUP_SBS):
            sb = g * GROUP_SBS + isb
            tsb, nch = t_sbs[sb], nchs[sb]
            ksb, npk = k_sbs[sb], np_sbs[sb]
            ns = min(SB, max(1, 512 // tsb))
            _mark(nc, 'dma_kt')
            if sb in kts:
                kt = kts[sb]
            else:
                kt = kt_pool.tile([H, SB * tsb], BF16, tag="kt", name=f"kt_{sb}")
                nc.sync.dma_start(kt[:],
                                  keysT_d[:, kt_offs[sb]:kt_offs[sb] + SB * tsb])
            _mark(nc, 'qk')

            def qbr(slot0, nsl, reps):
                # broadcast view of qt: slot columns repeated `reps` times
                return bass.AP(tensor=qt[:].tensor,
                               offset=sb * SB + slot0,
                               ap=[[qt_pitch, H], [1, nsl], [0, reps]])

            qkt = qk_pool.tile([H, SB * tsb], BF16, tag="qk", name=f"qk_{sb}")
            nc.vector.tensor_tensor(qkt[:], kt[:], qbr(0, SB, tsb), op=A.mult)
            scratch = scr_pool.tile([128, 4 * tsb], F32, tag="scr", name=f"scr_{sb}")
            ci = 0
            ps_s = None
            slot0 = 0
            while slot0 < SB:
                ns_c = min(ns, SB - slot0)
                cols = ns_c * tsb
                coff = slot0 * tsb
                _mark(nc, 'm1')
                ps1 = ps1_pool.tile([H1, cols], F32, tag="ps1", name=f"ps1_{sb}_{ci}")
                nc.tensor.matmul(ps1[:], wk[:], kt[:, coff:coff + cols],
                                 start=True, stop=False)
                nc.tensor.matmul(ps1[:], wqk[:], qkt[:, coff:coff + cols],
                                 start=False, stop=False)
                nc.tensor.matmul(ps1[:], wq[:], qbr(slot0, ns_c, tsb),
                                 start=False, stop=True)
                _mark(nc, 'relu1')
                h1 = h1_pool.tile([H1, cols], BF16, tag="h1", name=f"h1_{sb}_{ci}")
                nc.scalar.activation(h1[:], ps1[:], AF.Relu, bias=b1c[:, 0:1],
                                     scale=1.0)
                _mark(nc, 'm2')
                ps2 = ps2_pool.tile([H2, cols], F32, tag="ps2", name=f"ps2_{sb}_{ci}")
                nc.tensor.matmul(ps2[:], w2[:], h1[:], start=True, stop=True)
                _mark(nc, 'relu2')
                h2 = h2_pool.tile([H2, cols], BF16, tag="h2", name=f"h2_{sb}_{ci}")
                if (sb + ci) % 3 == 0:
                    nc.scalar.activation(h2[:], ps2[:], AF.Relu,
                                         bias=b2c[:, 0:1], scale=1.0)
                else:
                    nc.vector.tensor_scalar(h2[:], ps2[:], b2c[:, 0:1], 0.0,
                                            op0=A.add, op1=A.max)
                _mark(nc, 'm3')
                npq = 4 if tsb <= 128 else 2  # quads packed per scores bank
                for si in range(ns_c):
                    ssb = slot0 + si
                    q4, k4 = ssb // 4, ssb % 4
                    if ssb % (4 * npq) == 0:
                        ps_s = pss_t[0][:, 0:npq * tsb]
                    qq = q4 % npq
                    nc.tensor.matmul(ps_s[32 * k4:32 * k4 + 1,
                                          qq * tsb:(qq + 1) * tsb], wf[:],
                                     h2[:, si * tsb:(si + 1) * tsb],
                                     start=True, stop=True,
                                     tile_position=(0, 32 * k4),
                                     skip_group_check=True)
                    if ssb % (4 * npq) == 4 * npq - 1:
                        dst = scratch[:, (q4 - npq + 1) * tsb:(q4 + 1) * tsb]
                        if q4 % 2 == 0:
                            nc.scalar.copy(dst, ps_s)
                        else:
                            nc.vector.tensor_copy(dst, ps_s)
                slot0 += ns_c
                ci += 1
            _mark(nc, 'redis_s')
            # redistribute scratch -> scores rows [16isb, +16) with per-slot
            # column offset j*tsb (j = index within k-pack)
            sp = scratch[:].ap[0][0]
            P = scores[:].ap[0][0]
            for so, do, sdims, ddims in _redis_dmas(ksb, tsb, sp, P, npk):
                src = bass.AP(tensor=scratch[:].tensor,
                              offset=scratch[:].offset + so, ap=sdims)
                dst = bass.AP(tensor=scores[:].tensor,
                              offset=scores[:].offset + 16 * isb * P + do,
                              ap=ddims)
                nc.sync.dma_start(dst, src)
            _mark(nc, 'dma_kn')
            if tsb <= 128:
                knc = npk * 128
                kn = kn_pool.tile([128, knc], BF16, tag="kn", name=f"kn_{sb}")
                nc.sync.dma_start(kn[0:ksb * tsb, :],
                                  knat_d[0:ksb * tsb,
                                         kn_offs[sb]:kn_offs[sb] + knc])
            else:
                kn = kn_pool.tile([128, 4096], BF16, tag="kn", name=f"kn_{sb}")
                nc.sync.dma_start(kn[0:128, 0:2048],
                                  knat_d[0:128, kn_offs[sb]:kn_offs[sb] + 2048])
                nc.sync.dma_start(kn[0:tsb - 128, 2048:4096],
                                  knat_d[0:tsb - 128,
                                         kn_offs[sb] + 2048:kn_offs[sb] + 4096])
            knats[sb] = kn

        _mark(nc, 'softmax')
        # ---- batched softmax over the group ----
        # scaled scores are tiny (|s|<0.2 for randn inputs): no max-
        # subtraction needed; exp(NEG*SCALE) underflows to exactly 0.
        pexp = grp_pool.tile([gslots, tg], F32, tag="pexp", name=f"pexp_{g}")
        nc.scalar.activation(pexp[:], scores[:], AF.Exp, scale=SCALE)
        pm = grp_pool.tile([gslots, tg], F32, tag="pm", name=f"pm_{g}")
        nc.vector.tensor_tensor(pm[:], pexp[:], masks[g][:], op=A.mult)
        zsum = grp_pool.tile([gslots, 1], F32, tag="zsum", name=f"zsum_{g}")
        nc.vector.reduce_sum(zsum[:], pm[:], axis=mybir.AxisListType.X)
        rz = grp_pool.tile([gslots, 1], F32, tag="rz", name=f"rz_{g}")
        nc.vector.reciprocal(rz[:], zsum[:])
        attnb = grp_pool.tile([gslots, tg], BF16, tag="attnb", name=f"attnb_{g}")
        nc.vector.tensor_scalar_mul(attnb[:], pm[:], rz[:, 0:1])

        _mark(nc, 'transpose')
        # ---- transpose attn (bf16) in 128-col chunks ----
        atts = []
        for c in range(-(-tg // 128)):
            cl = min(128, tg - 128 * c)
            ps_t = pst_pool.tile([cl, gslots], BF16, tag="pst", name=f"pst_{g}_{c}")
            nc.tensor.transpose(ps_t[:], attnb[:, 128 * c:128 * c + cl],
                                identb[0:gslots, 0:gslots])
            at = at_pool.tile([cl, gslots], BF16, tag="at", name=f"at_{g}_{c}")
            nc.vector.tensor_copy(at[:], ps_t[:])
            atts.append(at)

        _mark(nc, 'final')
        # ---- final contraction: one matmul per k-pack ----
        for isb in range(GROUP_SBS):
            sb = g * GROUP_SBS + isb
            tsb, nch = t_sbs[sb], nchs[sb]
            ksb, npk = k_sbs[sb], np_sbs[sb]
            kn = knats[sb]
            ps_o = pso_t[sb % 2]
            at0 = atts[0]
            at_pitch = at0[:].ap[0][0]
            if ksb > 1:
                for p in range(npk):
                    lhsT = bass.AP(tensor=at0[:].tensor,
                                   offset=at0[:].offset + 16 * isb + p,
                                   ap=[[at_pitch, ksb * tsb], [npk, ksb]])
                    rhs = kn[0:ksb * tsb, p * 128:(p + 1) * 128]
                    po = ps_o[32 * (p % 4):32 * (p % 4) + ksb,
                              128 * (p // 4):128 * (p // 4) + 128]
                    nc.tensor.matmul(po, lhsT, rhs, start=True, stop=True,
                                     tile_position=(0, 32 * (p % 4)),
                                     skip_group_check=True)
            else:
                # 4 slots side-by-side in a 512-wide moving operand; slot
                # 4p+j sits at atts column 16isb + 4j + p (k=1 row perm)
                for p in range(4):
                    for c in range(nch):
                        cl = min(128, tsb - 128 * c)
                        atc = atts[c]
                        lhsT = bass.AP(tensor=atc[:].tensor,
                                       offset=atc[:].offset + 16 * isb + p,
                                       ap=[[atc[:].ap[0][0], cl], [4, 4]])
                        rhs = kn[0:cl, (c * 4 + p) * 512:(c * 4 + p + 1) * 512]
                        po = ps_o[32 * p:32 * p + 4, 0:512]
                        nc.tensor.matmul(po, lhsT, rhs,
                                         start=(c == 0), stop=(c == nch - 1),
                                         tile_position=(0, 32 * p),
                                         skip_group_check=True)
            # ---- evacuate + store: slot ssb = p*k + j ----
            used = 512 if ksb == 1 else 128 * -(-npk // 4)
            oscr = scr_pool.tile([128, used], F32, tag="oscr", name=f"oscr_{sb}")
            if isb % 2 == 0:
                nc.scalar.copy(oscr[:], ps_o[:, 0:used])
            else:
                nc.vector.tensor_copy(oscr[:], ps_o[:, 0:used])
            op = oscr[:].ap[0][0]
            if ksb > 1:
                # slot p*k+j at (partition 32*(p%4)+j, col 128*(p//4));
                # per-j DMAs (only dim 0 may step partitions)
                na, nb = max(1, npk // 4), min(npk, 4)
                sdims = [[32 * op, nb], [128, na], [1, 128]]
                ddims = [[ksb * 128, nb], [4 * ksb * 128, na], [1, 128]]
                if na == 1:
                    sdims = [sdims[0], sdims[2]]
                    ddims = [ddims[0], ddims[2]]
                dmas = [(j * op, j * 128, sdims, ddims) for j in range(ksb)]
            else:
                # slot 4p+j at (partition 32p+j, col 128j): per-j DMAs
                dmas = [(j * op + 128 * j, j * 128,
                         [[32 * op, 4], [1, 128]], [[4 * 128, 4], [1, 128]])
                        for j in range(4)]
            row0 = g * gslots + 16 * isb
            for so, do, sdims, ddims in dmas:
                src = bass.AP(tensor=oscr[:].tensor,
                              offset=oscr[:].offset + so, ap=sdims)
                dst = bass.AP(tensor=out_d.tensor,
                              offset=out_d.offset + row0 * 128 + do, ap=ddims)
                nc.gpsimd.dma_start(dst, src)


def pack_inputs(query, keys, keys_length, W1, b1, W2, b2, Wf, bf, plan):
    """Build the 8 per-core input maps."""
    bmap, t_sbs = plan["bmap"], plan["t_sbs"]
    k_sbs, np_sbs = plan["k_sbs"], plan["np_sbs"]
    kt_w, kn_w = plan["kt_w"], plan["kn_w"]
    Wq = (W1[0:H] + W1[2 * H:3 * H]).astype(np.float32)
    Wk = (W1[H:2 * H] - W1[2 * H:3 * H]).astype(np.float32)
    Wqk = W1[3 * H:4 * H].astype(np.float32)
    bft = ml_dtypes.bfloat16
    in_maps = []
    for c in range(NC):
        ktp = np.zeros((H, kt_w), bft)
        knp = np.zeros((128, kn_w), bft)
        qtp = np.zeros((H, SLOTS), bft)
        lensp = np.zeros((GROUP_SBS * SB, NGROUPS), np.float32)
        for sb in range(NSB):
            tsb = t_sbs[sb]
            ksb, npk = k_sbs[sb], np_sbs[sb]
            ko, no = plan["kt_offs"][sb], plan["kn_offs"][sb]
            g, isb = sb // GROUP_SBS, sb % GROUP_SBS
            for ssb in range(SB):
                s = sb * SB + ssb
                b = int(bmap[s, c])
                ktp[:, ko + ssb * tsb: ko + (ssb + 1) * tsb] = keys[b, :tsb, :].T
                p, j = ssb // ksb, ssb % ksb
                if tsb <= 128:
                    knp[j * tsb:(j + 1) * tsb, no + p * 128:no + (p + 1) * 128] = \
                        keys[b, :tsb, :]
                else:
                    pw, jw = ssb // 4, ssb % 4
                    nch = -(-tsb // 128)
                    for ch in range(nch):
                        cl = min(128, tsb - 128 * ch)
                        blk = no + (ch * 4 + pw) * 512 + jw * 128
                        knp[0:cl, blk:blk + 128] = keys[b, 128 * ch:128 * ch + cl, :]
                qtp[:, s] = query[b]
                lensp[16 * isb + _row_of(ssb, ksb, npk), g] = \
                    j * tsb + keys_length[b]
        in_maps.append({"keysT": ktp, "knat": knp, "queryT": qtp, "lens": lensp,
                        "wk": Wk.astype(bft), "wqk": Wqk.astype(bft),
                        "wq": Wq.astype(bft), "w2": W2.astype(bft),
                        "wf": Wf.astype(bft), "b1": b1.astype(np.float32),
                        "b2": b2.astype(np.float32)})
    return in_maps


def build_program(plan):
    nc = bacc.Bacc("TRN2", num_devices=NC)
    ins = [
        nc.dram_tensor("keysT", [H, plan["kt_w"]], BF16, kind="ExternalInput").ap(),
        nc.dram_tensor("knat", [128, plan["kn_w"]], BF16, kind="ExternalInput").ap(),
        nc.dram_tensor("queryT", [H, SLOTS], BF16, kind="ExternalInput").ap(),
        nc.dram_tensor("lens", [GROUP_SBS * SB, NGROUPS], F32,
                       kind="ExternalInput").ap(),
        nc.dram_tensor("wk", [H, H1], BF16, kind="ExternalInput").ap(),
        nc.dram_tensor("wqk", [H, H1], BF16, kind="ExternalInput").ap(),
        nc.dram_tensor("wq", [H, H1], BF16, kind="ExternalInput").ap(),
        nc.dram_tensor("w2", [H1, H2], BF16, kind="ExternalInput").ap(),
        nc.dram_tensor("wf", [H2, 1], BF16, kind="ExternalInput").ap(),
        nc.dram_tensor("b1", [H1], F32, kind="ExternalInput").ap(),
        nc.dram_tensor("b2", [H2], F32, kind="ExternalInput").ap(),
    ]
    outs = [nc.dram_tensor("outN", [SLOTS, H], F32, kind="ExternalOutput").ap()]
    with tile.TileContext(nc) as tc:
        with ExitStack() as ctx:
            build_body(ctx, tc, outs, ins, plan)
    nc.compile()
    return nc


last_results = None  # stash for external profiling/analysis


def kernel(query, keys, keys_length, W1, b1, W2, b2, Wf, bf):
    global last_results
    from concourse.bass_utils import run_bass_kernel_spmd
    query = np.asarray(query, np.float32)
    keys = np.asarray(keys, np.float32)
    keys_length = np.asarray(keys_length)
    plan = make_plan(keys_length)
    in_maps = pack_inputs(query, keys, keys_length, np.asarray(W1, np.float32),
                          np.asarray(b1, np.float32), np.asarray(W2, np.float32),
                          np.asarray(b2, np.float32), np.asarray(Wf, np.float32),
                          np.asarray(bf, np.float32), plan)
    nc = build_program(plan)
    trace = bool(int(os.environ.get("BASS_KERNEL_TRACE", "0")))
    res = run_bass_kernel_spmd(nc, in_maps, core_ids=list(range(NC)), trace=trace)
    last_results = res
    globals()["last_nc"] = nc
    if trace and res.exec_time_ns is not None:
        print(f"HW exec time: {res.exec_time_ns} ns")
    out = np.zeros((B, H), np.float32)
    bmap = plan["bmap"]
    for c in range(NC):
        outN = res.results[c]["outN"]  # [SLOTS, H], rows in slot order
        out[bmap[:, c]] = outN
    return out
